# revision 50
# baseline (speedup 1.0000x reference)
"""Trainium2 Bass kernel for nn_NoiseConv1d (channel-wise 6-bit quantize + 1x1 conv).

Math (per batch b, position n, with QL=63):
    cmax/cmin = max/min over channels c of x[b,:,n]
    s = (cmax-cmin)/QL ; q = round((x-cmin)/s)   (q in [0,63])
    out[o,n] = s[n] * sum_c W[o,c] q[c,n] + cmin[n]*Wsum[o] + bias[o]

Engine costs measured on HW via microbench.py (loop-slope op trains,
[128,512] fp16): DVE ts 212ns / +accum_out 623 / psum-in 457 (+accum 763);
ACT 659/753; Pool ts 699; PE tr 77, mm-512 246. The accum_out 3.2x penalty
(invisible to the cost model) makes DVE the pole engine; flags below were
chosen by same-run paired HW A/B to pack the quant passes across engines:
  - sj (batched [128,4] smalls, 5 DVE ops/chunk instead of 20)
  - dqpool (dequant on GPSIMD), qmix (q16 split ACT/GPSIMD by i-parity)
  - evmix (qt evacs split DVE/ACT), pairing ON, ld4 (quarter SWDGE loads)
  - qlag (optional): q-transposes+matmul lag one chunk so the in-order PE
    stream never parks on the quant-chain latency

v2 pipeline (one batch per NeuronCore, 8 cores data-parallel); historical
baseline mode was "v2_si_qact_evmix_nopair_warm" (96.7us its day; ~115us
re-measured 2026-08-11; the packed modes measure 87-90us same-run):
    - SWDGE cast-DMA: x fp32 HBM -> fp16 SBUF (natural [c, n] layout),
      staggered per n-chunk, chunk 0 split per-k so compute starts ~1.5us in
    - PE fp16 transpose 128x128 blocks -> psum xt [n, c] (1 cyc/row)
    - DVE fused evac+stat: xts = copy(xt) with accum max -> cmax; second
      tensor_scalar (4x fp16 sbuf) accum -> -cmin/63
    - per-i-tile smalls on [128,1]: s=d/63, r=1/s, B=-cmin*r+1536, c2=cmin-1536*s
    - ACT quant: q16 = RNE16(r*xts + B) (fp16 ulp == 1 in [1024,2048) ->
      exact integer+1536; ACT measured faster than DVE for this on HW)
    - DVE dequant+offset fold: qs' = q16*s + c2 == q*s + cmin  (so that
      out = W qs' + bias exactly -- no separate offset matmul, no hi/lo split)
    - PE fp16 transpose back to [c, n]; psum->SBUF evacs split DVE/ACT
    - PE matmul fp16 (psum f32, 512-wide rhs), ACT evac folds bias,
      fp16 output + host upcast (halves the store DMA)

Same-day paired loop-slope: v2 ~96.7us vs petr baseline ~117-126us (the
harness-quoted 94650ns petr baseline was measured on a faster tunnel day;
TimelineSim predicts v2 68.9us vs petr 84.7us).
"""

import sys

sys.path.insert(0, "/opt/trn_rl_repo")

import numpy as np

B, C, COUT, N = 8, 512, 512, 4096
QL = 63.0
MAGIC16 = 1536.0  # 1.5 * 2**10 : fp16 RNE round-to-int magic (ulp=1 in [1024,2048))

KT = C // 128  # 4 c-tiles
MT = COUT // 128  # 4 o-tiles
NT = N // 128  # 32 n-tiles
NCH = N // 512  # 8 n-chunks (psum free dim 512)

_cache = {}


def _build_v2(loop_n=0, mode="v2"):
    from contextlib import ExitStack

    import concourse.bass as bass
    import concourse.mybir as mybir
    from concourse import bacc
    from concourse.bass import ds, ts
    from concourse.masks import make_identity
    from concourse.tile import TileContext

    f32 = mybir.dt.float32
    f16 = mybir.dt.float16
    OP = mybir.AluOpType
    AF = mybir.ActivationFunctionType

    f32in = "f32in" in mode
    # odma: fold bias into the matmul via a 1-row ones matmul, then DMA the
    # PSUM tile straight to DRAM (f32) -- removes all 4 out-evac ACT passes
    # per chunk at the cost of 2x store bytes.
    odma = "odma" in mode
    f32out = "f32out" in mode or odma

    nc = bacc.Bacc(None, target_bir_lowering=False)
    xb = nc.dram_tensor("xb", [C, N], f32, kind="ExternalInput")
    wt = nc.dram_tensor("wt", [C, COUT], f16, kind="ExternalInput")  # W^T [c,o]
    bv = nc.dram_tensor("bv", [128, MT], f32, kind="ExternalInput")  # bias packed
    idm = nc.dram_tensor("idm", [128, 128], f16, kind="ExternalInput")  # np.eye
    if odma:
        br = nc.dram_tensor("br", [1, COUT], f16, kind="ExternalInput")
    out = nc.dram_tensor(
        "out", [COUT, N], f32 if f32out else f16, kind="ExternalOutput"
    )

    with ExitStack() as ctx:
        tc = ctx.enter_context(TileContext(nc))
        singles = ctx.enter_context(tc.tile_pool(name="singles", bufs=1))

        NQ = N // 4  # n-quarter size
        xdt = f32 if f32in else f16
        x_nat = [
            singles.tile([128, KT, NQ], xdt, name=f"x{q}", tag=f"x{q}")
            for q in range(4)
        ]
        wt_sb = singles.tile([128, KT, COUT], f16)
        bv_sb = singles.tile([128, MT], f32)
        ident16 = singles.tile([128, 128], f16)
        if f32in:
            ident32 = singles.tile([128, 128], f32)
            make_identity(nc, ident32)
        if odma:
            br_sb = singles.tile([1, COUT], f16)
            ones_row = singles.tile([1, 512], f16)
            nc.sync.dma_start(out=br_sb, in_=br[:, :])
            nc.vector.memset(ones_row, 1.0)

        nc.sync.dma_start(out=ident16, in_=idm[:, :])
        xq = xb.rearrange("(k p) n -> p k n", p=128)

        # fp16 psum tiles are half a bank; pair two i-tiles per tile so the
        # 8 psum banks hold a deeper pipeline (pairing only when fp16).
        nopair = "nopair" in mode or f32in
        ps_tr = ctx.enter_context(
            tc.tile_pool(name="ps_tr", bufs=3 if nopair else 2, space="PSUM")
        )
        ps_qt = ctx.enter_context(
            tc.tile_pool(name="ps_qt", bufs=2, space="PSUM")
        )
        ps_mm = ctx.enter_context(
            tc.tile_pool(
                name="ps_mm",
                bufs=4 if odma else (2 if f32in else (3 if nopair else 4)),
                space="PSUM",
            )
        )
        deep = 2 if "deep" in mode else 0  # deeper SBUF rings: smooths stalls
        stat = ctx.enter_context(tc.tile_pool(name="stat", bufs=5 + deep))
        xtsp = ctx.enter_context(tc.tile_pool(name="xtsp", bufs=3 + deep))
        qpool = ctx.enter_context(tc.tile_pool(name="qpool", bufs=8 + deep))
        jpool = ctx.enter_context(tc.tile_pool(name="jpool", bufs=3 + deep))
        opool = ctx.enter_context(tc.tile_pool(name="opool", bufs=6 + deep))
        qtpool = ctx.enter_context(tc.tile_pool(name="qtpool", bufs=4))

        per_i_smalls = "si" in mode

        def per_iter():
            if f32in:
                for q in range(4):
                    nc.sync.dma_start(out=x_nat[q], in_=xq[:, :, ds(q * NQ, NQ)])
                nc.sync.dma_start(
                    out=wt_sb, in_=wt.rearrange("(k p) o -> p k o", p=128)
                )
                nc.sync.dma_start(out=bv_sb, in_=bv[:, :])
            else:
                # SWDGE cast-DMA fp32 DRAM -> fp16 SBUF, staggered per chunk;
                # chunk 0 loads per-k (364ns first transfer) so transposes
                # unblock asap; weights/bias sequenced after chunk 0
                for k in range(KT):
                    nc.gpsimd.dma_start(
                        out=x_nat[0][:, k, ds(0, 512)],
                        in_=xq[:, k, ds(0, 512)],
                    )
                nc.sync.dma_start(
                    out=wt_sb, in_=wt.rearrange("(k p) o -> p k o", p=128)
                )
                nc.sync.dma_start(out=bv_sb, in_=bv[:, :])
                if "ld4" in mode:
                    # quarter-granular loads: fewer SWDGE desc-gen launches
                    nc.gpsimd.dma_start(
                        out=x_nat[0][:, :, ds(512, 512)],
                        in_=xq[:, :, ds(512, 512)],
                    )
                    for q in range(1, 4):
                        nc.gpsimd.dma_start(
                            out=x_nat[q], in_=xq[:, :, ds(q * NQ, NQ)]
                        )
                else:
                    for j in range(1, NCH):
                        nc.gpsimd.dma_start(
                            out=x_nat[j // 2][:, :, ds((j % 2) * 512, 512)],
                            in_=xq[:, :, ds(j * 512, 512)],
                        )

            xt_pair = [None]
            qt_pair = [None]

            if "warm" in mode:
                # dummy transposes bridge the initial DMA wait so the PE
                # p-state ramp overlaps the input load
                wps = ps_tr.tile([128, 2, 512], xdt, name="wps", tag="xt")
                for w_ in range(8):
                    nc.tensor.transpose(
                        wps[:, w_ % 2, ts(w_ % 4, 128)], ident16, ident16
                    )

            def smalls(mxc, ngc, w):
                # ngc holds -cmin/63 (prescaled in op2); returns r, B, s, c2
                sC = stat.tile([128, w], f32, tag="s4")
                rC = stat.tile([128, w], f32, tag="r4")
                BC = stat.tile([128, w], f32, tag="Bq")
                nC = stat.tile([128, w], f32, tag="ngr")
                cC = stat.tile([128, w], f32, tag="c24")
                nc.vector.scalar_tensor_tensor(
                    out=sC, in0=mxc, scalar=1.0 / QL, in1=ngc,
                    op0=OP.mult, op1=OP.add,
                )  # s = cmax/63 + (-cmin/63)
                nc.vector.reciprocal(rC, sC)  # r = 63/d
                nc.vector.scalar_tensor_tensor(
                    out=BC, in0=ngc, scalar=QL, in1=rC,
                    op0=OP.mult, op1=OP.mult,
                )  # -cmin * r
                nc.vector.tensor_scalar(
                    out=BC, in0=BC, scalar1=MAGIC16, scalar2=None, op0=OP.add
                )  # B = -cmin*r + 1536
                nc.vector.tensor_scalar_mul(nC, ngc, QL)  # -cmin
                nc.vector.scalar_tensor_tensor(
                    out=cC, in0=sC, scalar=-MAGIC16, in1=nC,
                    op0=OP.mult, op1=OP.subtract,
                )  # c2 = -1536*s - (-cmin) = cmin - 1536*s
                return rC, BC, sC, cC

            def quant_tail(xts, rcol, Bcol, scol, ccol, qsT, i):
                q16 = qpool.tile([128, 512], f16, tag="q16")
                if "qmix" in mode:
                    # q16 split ACT/Pool by i-parity (both measured ~660-700ns;
                    # relieves the ACT train, Pool has headroom)
                    if i % 2 == 0:
                        nc.scalar.activation(
                            out=q16, in_=xts, func=AF.Identity, bias=Bcol,
                            scale=rcol,
                        )
                    else:
                        nc.gpsimd.tensor_scalar(
                            out=q16, in0=xts, scalar1=rcol, scalar2=Bcol,
                            op0=OP.mult, op1=OP.add,
                        )
                elif "qact" in mode:
                    nc.scalar.activation(
                        out=q16, in_=xts, func=AF.Identity, bias=Bcol, scale=rcol
                    )
                elif "qpool" in mode:
                    nc.gpsimd.tensor_scalar(
                        out=q16, in0=xts, scalar1=rcol, scalar2=Bcol,
                        op0=OP.mult, op1=OP.add,
                    )
                else:
                    nc.vector.tensor_scalar(
                        out=q16, in0=xts, scalar1=rcol, scalar2=Bcol,
                        op0=OP.mult, op1=OP.add,
                    )  # fp16 write rounds to integer+1536
                qs = qpool.tile([128, 512], f16, tag="qs")
                if "qsact" in mode:
                    nc.scalar.activation(
                        out=qs, in_=q16, func=AF.Identity, bias=ccol, scale=scol
                    )
                elif "dqpool" in mode:
                    # dequant on GPSIMD (sbuf->sbuf, 699ns measured): DVE is
                    # the measured 9.7us/chunk pole, Pool is nearly idle
                    nc.gpsimd.tensor_scalar(
                        out=qs, in0=q16, scalar1=scol, scalar2=ccol,
                        op0=OP.mult, op1=OP.add,
                    )
                else:
                    nc.vector.tensor_scalar(
                        out=qs, in0=q16, scalar1=scol, scalar2=ccol,
                        op0=OP.mult, op1=OP.add,
                    )  # qs' = q*s + cmin  (offset folded)
                if "qlag" in mode:
                    return qs
                emit_qt(qs, qsT, i)
                return qs

            def emit_qt(qs, qsT, i):
                if nopair:
                    qt_pair[0] = ps_qt.tile([128, 1, KT * 128], f16, name="qtp", tag="qtp")
                    qt_ps = qt_pair[0][:, 0]
                else:
                    if i % 2 == 0:
                        qt_pair[0] = ps_qt.tile([128, 2, KT * 128], f16, name="qtp", tag="qtp")
                    qt_ps = qt_pair[0][:, i % 2]
                for k in range(KT):
                    nc.tensor.transpose(
                        qt_ps[:, ts(k, 128)], qs[:, ts(k, 128)], ident16
                    )
                ev_out = qsT[:, :, ds(i * 128, 128)]
                ev_in = qt_ps.rearrange("p (k n) -> p k n", k=KT)
                if "evdve" in mode:
                    nc.vector.tensor_copy(out=ev_out, in_=ev_in)
                elif "evpool" in mode:
                    nc.gpsimd.tensor_scalar_mul(ev_out, ev_in, 1.0)
                elif "ev1d" in mode and i == 0:
                    # only one of four qt evacs on DVE, rest ACT
                    nc.vector.tensor_copy(out=ev_out, in_=ev_in)
                elif "evmix" in mode and i % 2 == 0:
                    nc.vector.tensor_copy(out=ev_out, in_=ev_in)
                else:
                    nc.scalar.copy(out=ev_out, in_=ev_in)

            def mm_chunk(qsT, j):
                # ---- matmul for this n-chunk ----
                if "mmi" in mode:
                    # per-i-block matmuls: each 128-col psum region accumulates
                    # independently so blocks launch as soon as evac'd
                    pss = [ps_mm.tile([128, 512], f32, name="mm", tag="mm")
                           for _ in range(MT)]
                    if "mio" in mode:
                        order = [(m, i) for m in range(MT) for i in range(4)]
                    else:
                        order = [(m, i) for i in range(4) for m in range(MT)]
                    for m, i in order:
                        for k in range(KT):
                            nc.tensor.matmul(
                                pss[m][:, ds(i * 128, 128)],
                                wt_sb[:, k, ts(m, 128)],
                                qsT[:, k, ds(i * 128, 128)],
                                start=(k == 0),
                                stop=(k == KT - 1),
                            )
                else:
                    pss = []
                    for m in range(MT):
                        ps = ps_mm.tile([128, 512], f32, tag="mm")
                        for k in range(KT):
                            nc.tensor.matmul(
                                ps,
                                wt_sb[:, k, ts(m, 128)],
                                qsT[:, k, :],
                                start=(k == 0),
                                stop=(k == KT - 1) and not odma,
                            )
                        if odma:
                            # bias via a 1-row ones matmul: psum += 1 * bias[o]
                            nc.tensor.matmul(
                                ps,
                                br_sb[0:1, ts(m, 128)],
                                ones_row[0:1, :],
                                start=False,
                                stop=True,
                            )
                        pss.append(ps)
                for m in range(MT):
                    ps = pss[m]
                    ob = opool.tile([128, 512], f32 if f32out else f16, tag="ob")
                    if "odve" in mode or ("omix" in mode and m % 2 == 0):
                        nc.vector.tensor_scalar(
                            out=ob, in0=ps, scalar1=bv_sb[:, m : m + 1],
                            scalar2=None, op0=OP.add,
                        )
                    elif "opool" in mode or ("ophalf" in mode and m % 2 == 0):
                        nc.gpsimd.tensor_scalar(
                            out=ob, in0=ps, scalar1=bv_sb[:, m : m + 1],
                            scalar2=None, op0=OP.add,
                        )
                    else:
                        nc.scalar.activation(
                            out=ob, in_=ps, func=AF.Identity,
                            bias=bv_sb[:, m : m + 1], scale=1.0,
                        )
                    nc.sync.dma_start(
                        out=out[ts(m, 128), ds(512 * j, 512)], in_=ob
                    )


            # qlag: q-transposes + matmul of chunk j-1 are emitted during
            # chunk j, interleaved into the PE stream right after each x-
            # transpose. Without the lag, the in-order PE parks at qtr_i
            # until i's whole quant chain (~2.5us measured) finishes.
            qlag = "qlag" in mode
            pend = [None]  # (qs_list, qsT, j) of the previous chunk

            for j in range(NCH):
                if not per_i_smalls:
                    mx4 = stat.tile([128, 4], f32, tag="mx4")
                    ng4 = stat.tile([128, 4], f32, tag="ng4")
                qsT = qtpool.tile([128, KT, 512], f16, tag="qsT")
                xts_l = []
                for i in range(4):
                    t = 4 * j + i
                    # ---- transpose x block column t into [n, c] psum ----
                    if nopair:
                        xt_pair[0] = ps_tr.tile([128, 1, 512], xdt, name="xtp", tag="xt")
                        xt_ps = xt_pair[0][:, 0]
                    else:
                        if i % 2 == 0:
                            xt_pair[0] = ps_tr.tile([128, 2, 512], xdt, name="xtp", tag="xt")
                        xt_ps = xt_pair[0][:, i % 2]
                    xs = x_nat[t // 8]
                    tl = t % 8
                    for k in range(KT):
                        nc.tensor.transpose(
                            xt_ps[:, ts(k, 128)],
                            xs[:, k, ts(tl, 128)],
                            ident32 if f32in else ident16,
                        )
                    # ---- fused evac + max accumulate ----
                    if per_i_smalls:
                        mx4 = stat.tile([128, 1], f32, tag=f"mx{i}")
                        ng4 = stat.tile([128, 1], f32, tag=f"ng{i}")
                        mcol, ncol = mx4, ng4
                    else:
                        mcol = mx4[:, i : i + 1]
                        ncol = ng4[:, i : i + 1]
                    xts = xtsp.tile([128, 512], f16, tag=f"xts{i}")
                    nc.vector.tensor_scalar(
                        out=xts, in0=xt_ps, scalar1=1.0, scalar2=None,
                        op0=OP.mult, op1=OP.max, accum_out=mcol,
                    )
                    junk = jpool.tile([128, 512], f16, tag="junk")
                    eng2 = nc.gpsimd if "o2pool" in mode else nc.vector
                    eng2.tensor_scalar(
                        out=junk, in0=xts, scalar1=-1.0 / QL, scalar2=None,
                        op0=OP.mult, op1=OP.max, accum_out=ncol,
                    )  # accumulates -cmin/63
                    if qlag and pend[0] is not None:
                        # interleave the lagged chunk's q-transpose i into
                        # the PE stream right behind this x-transpose
                        emit_qt(pend[0][0][i], pend[0][1], i)
                    if per_i_smalls:
                        rC, BC, sC, cC = smalls(mx4, ng4, 1)
                        quant_tail(xts, rC, BC, sC, cC, qsT, i)
                    else:
                        xts_l.append(xts)

                if not per_i_smalls:
                    rC, BC, sC, cC = smalls(mx4, ng4, 4)
                    qs_list = []
                    for i in range(4):
                        qs_list.append(quant_tail(
                            xts_l[i], rC[:, i : i + 1], BC[:, i : i + 1],
                            sC[:, i : i + 1], cC[:, i : i + 1], qsT, i,
                        ))

                if qlag:
                    prev = pend[0]
                    pend[0] = (qs_list, qsT, j)
                    if prev is not None:
                        mm_chunk(prev[1], prev[2])
                    if j == NCH - 1:
                        for i in range(4):
                            emit_qt(qs_list[i], qsT, i)
                        mm_chunk(qsT, j)
                    continue
                mm_chunk(qsT, j)

        if loop_n:
            with tc.For_i(0, loop_n, 1):
                per_iter()
        else:
            per_iter()

    nc.compile()
    return nc


def _prep_weights_v2(weight, bias):
    W = weight[:, :, 0].astype(np.float64)  # [o, c]
    wt = np.ascontiguousarray(W.T).astype(np.float16)  # [c, o]
    bv = np.ascontiguousarray(bias.reshape(MT, 128).T).astype(np.float32)  # [128, MT]
    idm = np.eye(128, dtype=np.float16)
    return wt, bv, idm


def _build_v3(loop_n=0, mode="v3"):
    """v3: elementwise passes rebalanced off ACT + XBAR DMA q-transpose.

    Cost-model findings driving this (instruction_cost_v2.rs):
      - DVE tensor_scalar fp16/SBUF runs in 4x mode (~194ns per [128,512]
        pass); ACT has no fast mode (~570-612ns). v2 put the q16 quant on
        ACT (4x612ns/chunk), making ACT the 50us bottleneck engine.
      - InstDmaTransposeAnt (XBAR) costs 14ns per 16x128 tile: one
        [128,2048] fp16 transpose per n-chunk = 1.8us on the DMA engines
        and ~0.6us HWDGE, replacing 16 PE transposes AND the 4 PSUM->SBUF
        evac passes per chunk.

    Pipeline per i-tile (128 positions), all quant math on DVE:
      PE  x-transpose -> psum [n, c]
      DVE xts = copy(psum) accum-max -> cmax          (392ns, psum 2x)
      DVE junk = xts * (-1/63) accum-max -> -cmin/63  (194ns, 4x)
      DVE q16 = RNE16(r*xts + B)                      (194ns)
      DVE qs[:, i] = q16*s + c2                       (194ns)
    Per chunk: one XBAR transpose qs [128,(4,512)] -> qsT [128,(4,4),128]
    (out[c%128, i, k, n] = qs[n, i, 128k+c%128]), then 4 matmuls with
    rhs = qsT[:, :, k, :], ACT evac folds bias, fp16 store.
    """
    from contextlib import ExitStack

    import concourse.bass as bass
    import concourse.mybir as mybir
    from concourse import bacc
    from concourse.bass import ds, ts
    from concourse.tile import TileContext

    f32 = mybir.dt.float32
    f16 = mybir.dt.float16
    OP = mybir.AluOpType
    AF = mybir.ActivationFunctionType

    nc = bacc.Bacc(None, target_bir_lowering=False)
    xb = nc.dram_tensor("xb", [C, N], f32, kind="ExternalInput")
    wt = nc.dram_tensor("wt", [C, COUT], f16, kind="ExternalInput")  # W^T [c,o]
    bv = nc.dram_tensor("bv", [128, MT], f32, kind="ExternalInput")  # bias packed
    idm = nc.dram_tensor("idm", [128, 128], f16, kind="ExternalInput")  # np.eye
    out = nc.dram_tensor("out", [COUT, N], f16, kind="ExternalOutput")

    with ExitStack() as ctx:
        tc = ctx.enter_context(TileContext(nc))
        singles = ctx.enter_context(tc.tile_pool(name="singles", bufs=1))

        NQ = N // 4
        x_nat = [
            singles.tile([128, KT, NQ], f16, name=f"x{q}", tag=f"x{q}")
            for q in range(4)
        ]
        wt_sb = singles.tile([128, KT, COUT], f16)
        bv_sb = singles.tile([128, MT], f32)
        ident16 = singles.tile([128, 128], f16)
        nc.sync.dma_start(out=ident16, in_=idm[:, :])
        xq = xb.rearrange("(k p) n -> p k n", p=128)

        ps_tr = ctx.enter_context(tc.tile_pool(name="ps_tr", bufs=3, space="PSUM"))
        ps_mm = ctx.enter_context(tc.tile_pool(name="ps_mm", bufs=4, space="PSUM"))
        stat = ctx.enter_context(tc.tile_pool(name="stat", bufs=5))
        xtsp = ctx.enter_context(tc.tile_pool(name="xtsp", bufs=3))
        qpool = ctx.enter_context(tc.tile_pool(name="qpool", bufs=4))
        jpool = ctx.enter_context(tc.tile_pool(name="jpool", bufs=3))
        opool = ctx.enter_context(tc.tile_pool(name="opool", bufs=6))
        qspool = ctx.enter_context(tc.tile_pool(name="qspool", bufs=3))
        qtpool = ctx.enter_context(
            tc.tile_pool(name="qtpool", bufs=5 if "lag3" in mode else 4)
        )

        dq_act = "dqact" in mode  # dequant pass on ACT instead of DVE
        ev_dve = "odve" in mode  # out evac on DVE instead of ACT

        def per_iter():
            # SWDGE cast-DMA fp32 DRAM -> fp16 SBUF, chunk 0 per-k so the
            # first transposes unblock early (as v2)
            for k in range(KT):
                nc.gpsimd.dma_start(
                    out=x_nat[0][:, k, ds(0, 512)], in_=xq[:, k, ds(0, 512)]
                )
            nc.sync.dma_start(
                out=wt_sb, in_=wt.rearrange("(k p) o -> p k o", p=128)
            )
            nc.sync.dma_start(out=bv_sb, in_=bv[:, :])
            for j in range(1, NCH):
                nc.gpsimd.dma_start(
                    out=x_nat[j // 2][:, :, ds((j % 2) * 512, 512)],
                    in_=xq[:, :, ds(j * 512, 512)],
                )

            if "warm" in mode:
                wps = ps_tr.tile([128, 512], f16, name="wps", tag="xt")
                for w_ in range(8):
                    nc.tensor.transpose(
                        wps[:, ts(w_ % 4, 128)], ident16, ident16
                    )

            # Software-pipelined across chunks: PE program order is
            # [x-tr j][mm j-1][x-tr j+1][mm j]... so the in-order PE stream
            # never parks on chunk j's XBAR-transpose wait (which would both
            # stall the pipeline and drop PE out of its high p-state).
            def mm_one(j, qsT, m):
                # one m-tile of chunk j's matmul + evac + store
                ps = ps_mm.tile([128, 512], f32, name="mm", tag="mm")
                for k in range(KT):
                    nc.tensor.matmul(
                        ps,
                        wt_sb[:, k, ts(m, 128)],
                        qsT[:, :, k, :],
                        start=(k == 0),
                        stop=(k == KT - 1),
                    )
                ob = opool.tile([128, 512], f16, tag="ob")
                if ev_dve:
                    nc.vector.tensor_scalar(
                        out=ob, in0=ps, scalar1=bv_sb[:, m : m + 1],
                        scalar2=None, op0=OP.add,
                    )
                else:
                    nc.scalar.activation(
                        out=ob, in_=ps, func=AF.Identity,
                        bias=bv_sb[:, m : m + 1], scale=1.0,
                    )
                nc.sync.dma_start(
                    out=out[ts(m, 128), ds(512 * j, 512)], in_=ob
                )

            def smalls(mxc, ngc, w):
                # ngc holds -cmin/63; returns r, B, s, c2 as [128, w]
                sC = stat.tile([128, w], f32, tag="s4")
                rC = stat.tile([128, w], f32, tag="r4")
                BC = stat.tile([128, w], f32, tag="Bq")
                nC = stat.tile([128, w], f32, tag="ngr")
                cC = stat.tile([128, w], f32, tag="c24")
                nc.vector.scalar_tensor_tensor(
                    out=sC, in0=mxc, scalar=1.0 / QL, in1=ngc,
                    op0=OP.mult, op1=OP.add,
                )  # s = cmax/63 + (-cmin/63)
                nc.vector.reciprocal(rC, sC)
                nc.vector.scalar_tensor_tensor(
                    out=BC, in0=ngc, scalar=QL, in1=rC,
                    op0=OP.mult, op1=OP.mult,
                )  # -cmin * r
                nc.vector.tensor_scalar(
                    out=BC, in0=BC, scalar1=MAGIC16, scalar2=None, op0=OP.add
                )  # B = -cmin*r + 1536
                nc.vector.tensor_scalar_mul(nC, ngc, QL)  # -cmin
                nc.vector.scalar_tensor_tensor(
                    out=cC, in0=sC, scalar=-MAGIC16, in1=nC,
                    op0=OP.mult, op1=OP.subtract,
                )  # c2 = cmin - 1536*s
                return rC, BC, sC, cC

            sj = "sj" in mode  # smalls batched per chunk
            dq_mix = "dqmix" in mode  # dequant alternates DVE/ACT

            def quant_tail(xts, rcol, Bcol, scol, ccol, qs_all, i):
                q16 = qpool.tile([128, 512], f16, tag="q16")
                nc.vector.tensor_scalar(
                    out=q16, in0=xts, scalar1=rcol, scalar2=Bcol,
                    op0=OP.mult, op1=OP.add,
                )  # fp16 write rounds to integer+1536
                if dq_act or (dq_mix and i % 2 == 1):
                    nc.scalar.activation(
                        out=qs_all[:, i], in_=q16, func=AF.Identity,
                        bias=ccol, scale=scol,
                    )
                else:
                    nc.vector.tensor_scalar(
                        out=qs_all[:, i], in0=q16, scalar1=scol, scalar2=ccol,
                        op0=OP.mult, op1=OP.add,
                    )  # qs = q*s + cmin

            # mm lags the quant pipeline by `lag` chunks so the XBAR transpose
            # (~2.7us issue-to-sem latency) is fully hidden; PE interleaves
            # [tr_i of chunk j][mm_m of chunk j-lag] to stay continuously busy
            # (the cost model's p-state ramp doubles matmul cost if PE idles).
            lag = 1 if "lag1" in mode else (3 if "lag3" in mode else 2)
            pending = []  # [(j, qsT)] chunks awaiting their matmul block

            for j in range(NCH):
                qs_all = qspool.tile([128, 4, 512], f16, tag="qs")
                if sj:
                    mx4 = stat.tile([128, 4], f32, tag="mx4")
                    ng4 = stat.tile([128, 4], f32, tag="ng4")
                    xts_l = []
                for i in range(4):
                    t = 4 * j + i
                    xt_ps = ps_tr.tile([128, 512], f16, name="xtp", tag="xt")
                    xs = x_nat[t // 8]
                    tl = t % 8
                    for k in range(KT):
                        nc.tensor.transpose(
                            xt_ps[:, ts(k, 128)], xs[:, k, ts(tl, 128)], ident16
                        )
                    if sj:
                        mx = mx4[:, i : i + 1]
                        ng = ng4[:, i : i + 1]
                    else:
                        mx = stat.tile([128, 1], f32, name="mx", tag=f"mx{i}")
                        ng = stat.tile([128, 1], f32, name="ng", tag=f"ng{i}")
                    xts = xtsp.tile([128, 512], f16, tag=f"xts{i}")
                    nc.vector.tensor_scalar(
                        out=xts, in0=xt_ps, scalar1=1.0, scalar2=None,
                        op0=OP.mult, op1=OP.max, accum_out=mx,
                    )
                    junk = jpool.tile([128, 512], f16, tag="junk")
                    nc.vector.tensor_scalar(
                        out=junk, in0=xts, scalar1=-1.0 / QL, scalar2=None,
                        op0=OP.mult, op1=OP.max, accum_out=ng,
                    )  # accumulates -cmin/63
                    if sj:
                        xts_l.append(xts)
                    else:
                        rC, BC, sC, cC = smalls(mx, ng, 1)
                        quant_tail(xts, rC, BC, sC, cC, qs_all, i)

                if sj:
                    rC, BC, sC, cC = smalls(mx4, ng4, 4)
                    for i in range(4):
                        quant_tail(
                            xts_l[i], rC[:, i : i + 1], BC[:, i : i + 1],
                            sC[:, i : i + 1], cC[:, i : i + 1], qs_all, i,
                        )

                # one XBAR transpose for the whole chunk:
                # qsT[c%128, i, k, n128] = qs_all[n128, i, 128k + c%128]
                # Issued on the DVE queue: its only dependency (qs_all) is
                # the op right before it there, so it can never be head-of-
                # line blocked behind work that waits on downstream results
                # (stores on SP wait on evacs; evacs on ACT wait on matmuls).
                qsT = qtpool.tile([128, 4, KT, 128], f16, tag="qsT")
                nc.scalar.dma_start_transpose(
                    out=qsT, in_=qs_all.rearrange("p a b -> p (a b)")
                )
                pending.append((j, qsT))
                # Drain the mm block of chunk j-lag AFTER chunk j's XBAR, so
                # on the in-order ACT queue each XBAR sits only behind evacs
                # whose matmuls completed a chunk ago (true deps only), and
                # the PE stream alternates [tr j][mm j-lag] with no hole.
                if len(pending) > lag:
                    jd, qsTd = pending.pop(0)
                    for m in range(MT):
                        mm_one(jd, qsTd, m)
            for jd, qsTd in pending:
                for m in range(MT):
                    mm_one(jd, qsTd, m)

        if loop_n:
            with tc.For_i(0, loop_n, 1):
                per_iter()
        else:
            per_iter()

    nc.compile()
    return nc


import os as _os

MODE = _os.environ.get("BASS_MODE", "v2_qmix_evmix_warm_dqpool_ld4")


def kernel(x, weight, bias):
    from concourse.bass_utils import run_bass_kernel_spmd

    mode = _cache.get("mode", MODE)
    if "nc" not in _cache:
        _cache["nc"] = _build_bass(mode=mode)
    nc = _cache["nc"]

    wt, bv, idm = _prep_weights_v2(np.asarray(weight), np.asarray(bias))
    x = np.asarray(x, dtype=np.float32)
    base = {"wt": wt, "bv": bv, "idm": idm}
    if "odma" in mode:
        base["br"] = np.asarray(bias, dtype=np.float16).reshape(1, COUT)
    in_maps = [
        {"xb": np.ascontiguousarray(x[i]), **base} for i in range(B)
    ]
    res = run_bass_kernel_spmd(nc, in_maps, core_ids=list(range(B)))
    return np.stack([r["out"] for r in res.results], axis=0).astype(np.float32)


# ---------------------------------------------------------------------------
# Previous-generation builder ("petr" baseline) kept for A/B reference.
# ---------------------------------------------------------------------------


def _prep_weights(weight, bias):
    W = weight[:, :, 0].astype(np.float64)  # [o, c]
    wt = np.ascontiguousarray(W.T).astype(np.float16)  # [c, o]
    wsum = W.sum(axis=1)  # [o]
    whi = wsum.astype(np.float16)
    wr = np.stack([whi, whi], axis=0)  # [2, o]
    bv = np.ascontiguousarray(bias.reshape(MT, 128).T).astype(np.float32)
    return wt, wr, bv


def _build_bass(loop_n=0, mode="full"):
    from contextlib import ExitStack

    import concourse.bass as bass
    import concourse.mybir as mybir
    from concourse import bacc
    from concourse.bass import ds, ts
    from concourse.masks import make_identity
    from concourse.tile import TileContext

    if mode.startswith("v3"):
        return _build_v3(loop_n=loop_n, mode=mode)
    if mode.startswith("v2"):
        return _build_v2(loop_n=loop_n, mode=mode)

    f32 = mybir.dt.float32
    f16 = mybir.dt.float16
    AX = mybir.AxisListType
    OP = mybir.AluOpType
    AF = mybir.ActivationFunctionType

    MAGIC = float(np.float32(12582912.0))

    nc = bacc.Bacc(None, target_bir_lowering=False)
    xb = nc.dram_tensor("xb", [C, N], f32, kind="ExternalInput")
    wt = nc.dram_tensor("wt", [C, COUT], f16, kind="ExternalInput")  # W^T [c,o]
    wr = nc.dram_tensor("wr", [2, COUT], f16, kind="ExternalInput")  # [Whi,Whi]
    bv = nc.dram_tensor("bv", [128, MT], f32, kind="ExternalInput")  # bias packed
    out = nc.dram_tensor("out", [COUT, N], f32, kind="ExternalOutput")

    with ExitStack() as ctx:
        tc = ctx.enter_context(TileContext(nc))
        singles = ctx.enter_context(tc.tile_pool(name="singles", bufs=1))

        NQ = N // 4  # n-quarter size
        if "cf16" in mode:
            x_nat = [
                singles.tile([128, KT, NQ], f16, name=f"x{q}", tag=f"x{q}")
                for q in range(4)
            ]
        else:
            x_nat = [
                singles.tile([128, KT, NQ], f32, name=f"x{q}", tag=f"x{q}")
                for q in range(4)
            ]
        wt_sb = singles.tile([128, KT, COUT], f16)
        wr_sb = singles.tile([2, COUT], f16)
        bv_sb = singles.tile([128, MT], f32)
        ident = singles.tile([128, 128], f32)
        ident16 = singles.tile([128, 128], f16)

        make_identity(nc, ident)
        make_identity(nc, ident16)
        nc.sync.dma_start(out=wt_sb, in_=wt.rearrange("(k p) o -> p k o", p=128))
        nc.sync.dma_start(out=wr_sb, in_=wr[:, :])
        nc.sync.dma_start(out=bv_sb, in_=bv[:, :])
        xq = xb.rearrange("(k p) n -> p k n", p=128)

        ps_tr = ctx.enter_context(tc.tile_pool(name="ps_tr", bufs=3, space="PSUM"))
        ps_qt = ctx.enter_context(tc.tile_pool(name="ps_qt", bufs=2, space="PSUM"))
        ps_mm = ctx.enter_context(tc.tile_pool(name="ps_mm", bufs=3, space="PSUM"))
        stat = ctx.enter_context(tc.tile_pool(name="stat", bufs=12))
        qpool = ctx.enter_context(tc.tile_pool(name="qpool", bufs=6))
        jpool = ctx.enter_context(tc.tile_pool(name="jpool", bufs=4))
        opool = ctx.enter_context(tc.tile_pool(name="opool", bufs=6))
        qtpool = ctx.enter_context(tc.tile_pool(name="qtpool", bufs=4))

        def per_iter():
            for q in range(4):
                if "cf16" in mode:
                    nc.gpsimd.dma_start(out=x_nat[q], in_=xq[:, :, ds(q * NQ, NQ)])
                else:
                    nc.sync.dma_start(out=x_nat[q], in_=xq[:, :, ds(q * NQ, NQ)])
            body_chunks()

        def body_chunks():
          for j in range(NCH):
            qsT = qtpool.tile([128, 4, KT, 128], f16, tag="qsT")
            cmr = qtpool.tile([2, 4, 128], f16, tag="cmr")
            hl = stat.tile([128, 2, 4], f16, tag="hl")
            for i in range(4):
                t = 4 * j + i
                cf = "cf16" in mode
                xt_ps = ps_tr.tile([128, 512], f16 if cf else f32, tag="xt")
                xs = x_nat[t // 8]
                tl = t % 8
                for k in range(KT):
                    nc.tensor.transpose(
                        xt_ps[:, ts(k, 128)],
                        xs[:, k, ts(tl, 128)],
                        ident16 if cf else ident,
                    )
                xts_t = qpool.tile([128, 512], f16 if cf else f32, tag="xts")
                nc.scalar.copy(out=xts_t, in_=xt_ps)
                xts = xts_t
                mx = stat.tile([128, 1], f32, tag="mx")
                ng = stat.tile([128, 1], f32, tag="ng")
                junk = jpool.tile([128, 512], f16, tag="junk")
                nc.vector.tensor_scalar(
                    out=junk, in0=xts, scalar1=1.0, scalar2=None,
                    op0=OP.mult, op1=OP.max, accum_out=mx,
                )
                nc.vector.tensor_scalar(
                    out=junk, in0=xts, scalar1=-1.0, scalar2=None,
                    op0=OP.mult, op1=OP.max, accum_out=ng,
                )
                d = stat.tile([128, 1], f32, tag="d")
                s = stat.tile([128, 1], f32, tag="s")
                r = stat.tile([128, 1], f32, tag="r")
                Bt = stat.tile([128, 1], f32, tag="Bt")
                nc.vector.tensor_add(d, mx, ng)
                nc.vector.tensor_scalar_mul(s, d, 1.0 / QL)
                nc.vector.reciprocal(r, s)
                nc.vector.tensor_scalar(
                    out=Bt, in0=ng, scalar1=r, scalar2=MAGIC16,
                    op0=OP.mult, op1=OP.add,
                )
                qp = qpool.tile([128, 512], f16, tag="qp")
                nc.scalar.activation(
                    out=qp, in_=xts, func=AF.Identity, bias=Bt, scale=r
                )
                qs = qpool.tile([128, KT * 128], f16, tag="qs")
                nc.vector.tensor_scalar(
                    out=qs, in0=qp, scalar1=MAGIC16, scalar2=s,
                    op0=OP.subtract, op1=OP.mult,
                )
                nc.gpsimd.tensor_scalar_mul(hl[:, 0, i : i + 1], ng, -1.0)
                nc.gpsimd.tensor_scalar(
                    out=hl[:, 1, i : i + 1], in0=hl[:, 0, i : i + 1],
                    scalar1=ng, scalar2=-1.0, op0=OP.add, op1=OP.mult,
                )
                qt_ps = ps_qt.tile([128, (KT + 1) * 128], f16, tag="qtp")
                for b_ in range(KT):
                    nc.tensor.transpose(
                        qt_ps[:, ts(b_, 128)], qs[:, ts(b_, 128)], ident16
                    )
                nc.tensor.transpose(
                    qt_ps[0:2, ts(KT, 128)], hl[:, :, i], ident16
                )
                nc.vector.tensor_copy(out=qsT[:, i], in_=qt_ps[:, 0 : KT * 128])
                nc.vector.tensor_copy(
                    out=cmr[:, i], in_=qt_ps[0:2, ts(KT, 128)]
                )

            for m in range(MT):
                ps = ps_mm.tile([128, 512], f32, tag="mm")
                for k in range(KT):
                    nc.tensor.matmul(
                        ps,
                        wt_sb[:, k, ts(m, 128)],
                        qsT[:, :, k, :],
                        start=(k == 0),
                        stop=False,
                    )
                nc.tensor.matmul(
                    ps,
                    wr_sb[:, ts(m, 128)],
                    cmr[:, :, :],
                    start=False,
                    stop=True,
                )
                ob = opool.tile([128, 512], f32, tag="ob")
                nc.scalar.activation(
                    out=ob, in_=ps, func=AF.Identity,
                    bias=bv_sb[:, m : m + 1], scale=1.0,
                )
                nc.sync.dma_start(
                    out=out[ts(m, 128), ds(512 * j, 512)], in_=ob
                )

        if loop_n:
            with tc.For_i(0, loop_n, 1):
                per_iter()
        else:
            per_iter()

    nc.compile()
    return nc

